# revision 2
# baseline (speedup 1.0000x reference)
"""Chamfer-distance-with-normals Trainium2 kernel.

Sharding: data-parallel over batch B=8 across the 8 NeuronCores (one batch
element per core). Per core, the 4096x4096 squared-distance matrix is tiled
as 32 row-blocks x 8 column-tiles of [128, 512], produced by K=5 matmuls
that fold both squared-norm rank-1 terms into the contraction:

    W[n, m] = sum_k lhsT[k, n] * rhs[k, m]
            = 2*x1.x2 - |x2|^2 - |x1|^2  =  -D[n, m]

so row-max of W == -min-dist, and argmax == argmin. Row max/argmax come from
the DVE max8/max_index instructions over the full 4096-wide row resident in
SBUF (ScalarE drains PSUM banks into SBUF). Two passes: [n,m] orientation for
dist1/idx1 and [m,n] for dist2/idx2. The O(B*N) normal gather + normalize +
mean runs on host in float64.
"""

import functools
from contextlib import ExitStack

import numpy as np

import concourse.bass as bass
import concourse.mybir as mybir
import concourse.tile as tile
from concourse import bacc
from concourse.bass_utils import run_bass_kernel_spmd

B = 8
N_PTS = 4096
M_TILE = 512  # fp32 moving-operand max / one PSUM bank
P = 128


def build_chamfer(n_pts=N_PTS, m_tile=M_TILE, psum_halves=2):
    """Build the Bass program. Returns (nc, io_names)."""
    nc = bacc.Bacc("TRN2", target_bir_lowering=False, debug=False, num_devices=B)
    dt = mybir.dt

    n_blocks = n_pts // P
    psum_halves = min(psum_halves, n_pts // m_tile)
    # PSUM tile = `psum_halves` banks wide; 8/psum_halves tiles in flight.
    psum_w = psum_halves * m_tile
    n_chunks = n_pts // psum_w  # SBUF-row chunks per block

    # Matmul operand matrices, built on host (see kernel()).
    ins = {}
    for name in ("s1", "t2", "s2", "t1"):
        ins[name] = nc.dram_tensor(name, [5, n_pts], dt.float32, kind="ExternalInput").ap()
    outs = {}
    for name in ("d1", "d2"):
        outs[name] = nc.dram_tensor(name, [P, n_blocks * 8], dt.float32, kind="ExternalOutput").ap()
    for name in ("i1", "i2"):
        outs[name] = nc.dram_tensor(name, [P, n_blocks * 8], dt.uint32, kind="ExternalOutput").ap()

    with tile.TileContext(nc) as tc, ExitStack() as ctx:
        const_pool = ctx.enter_context(tc.tile_pool(name="const", bufs=1))
        w_pool = ctx.enter_context(tc.tile_pool(name="wrow", bufs=2))
        psum_pool = ctx.enter_context(
            tc.tile_pool(name="psum", bufs=8 // psum_halves, space="PSUM")
        )
        res_pool = ctx.enter_context(tc.tile_pool(name="res", bufs=1))

        # Load all four operand matrices to SBUF.
        op_tiles = {}
        for name in ("s1", "t2", "s2", "t1"):
            t = const_pool.tile([5, n_pts], dt.float32, tag=name)
            nc.sync.dma_start(t[:], ins[name][:])
            op_tiles[name] = t

        for pno, (lhs_name, rhs_name, dname, iname) in enumerate(
            (("s1", "t2", "d1", "i1"), ("s2", "t1", "d2", "i2"))
        ):
            lhs = op_tiles[lhs_name]
            rhs = op_tiles[rhs_name]
            vals = res_pool.tile([P, n_blocks * 8], dt.float32, tag=f"v{pno}")
            idxs = res_pool.tile([P, n_blocks * 8], dt.uint32, tag=f"x{pno}")

            for i in range(n_blocks):
                w_sb = w_pool.tile([P, n_pts], dt.float32, tag="w")
                for h in range(n_chunks):
                    ps = psum_pool.tile([P, psum_w], dt.float32, tag="ps")
                    for jj in range(psum_halves):
                        j = psum_halves * h + jj
                        nc.tensor.matmul(
                            ps[:, jj * m_tile : (jj + 1) * m_tile],
                            lhs[:, i * P : (i + 1) * P],
                            rhs[:, j * m_tile : (j + 1) * m_tile],
                            start=True,
                            stop=True,
                        )
                    nc.scalar.copy(w_sb[:, h * psum_w : (h + 1) * psum_w], ps[:])
                nc.vector.max(vals[:, i * 8 : (i + 1) * 8], w_sb[:])
                nc.vector.max_index(idxs[:, i * 8 : (i + 1) * 8], vals[:, i * 8 : (i + 1) * 8], w_sb[:])

            nc.sync.dma_start(outs[dname][:], vals[:])
            nc.sync.dma_start(outs[iname][:], idxs[:])

    nc.compile()
    return nc


@functools.lru_cache(maxsize=1)
def _compiled():
    return build_chamfer()


def _operands(xyz):
    """[n,3] fp32 -> (S, T) [5,n] matmul operand rows (see module docstring)."""
    n = xyz.shape[0]
    x, y, z = xyz[:, 0], xyz[:, 1], xyz[:, 2]
    sq = x * x + y * y + z * z  # fp32
    ones = np.ones(n, np.float32)
    s = np.stack([x, y, z, ones, -sq]).astype(np.float32, copy=False)
    t = np.stack([2 * x, 2 * y, 2 * z, -sq, ones]).astype(np.float32, copy=False)
    return np.ascontiguousarray(s), np.ascontiguousarray(t)


def _unpack(res, n_pts):
    """Device outputs [128, blocks*8] slot-0-of-8 -> flat [n] arrays."""
    n_blocks = n_pts // P
    d = {}
    for name in ("d1", "d2", "i1", "i2"):
        a = res[name].reshape(P, n_blocks, 8)[:, :, 0]  # [128, blocks]
        d[name] = np.ascontiguousarray(a.T).reshape(-1)  # n = 128*block + p
    return d


def kernel(xyz1, xyz2, normal_rebuild, normal_gt):
    nc = _compiled()

    in_maps = []
    for b in range(B):
        s1, t1 = _operands(np.asarray(xyz1[b], np.float32))
        s2, t2 = _operands(np.asarray(xyz2[b], np.float32))
        in_maps.append({"s1": s1, "t1": t1, "s2": s2, "t2": t2})

    res = run_bass_kernel_spmd(nc, in_maps, core_ids=list(range(B)))

    loss_xyz = 0.0
    loss_normal = 0.0
    for b in range(B):
        r = _unpack(res.results[b], N_PTS)
        dist1, dist2 = -r["d1"].astype(np.float64), -r["d2"].astype(np.float64)
        idx1, idx2 = r["i1"], r["i2"]
        loss_xyz += dist1.mean() + dist2.mean()

        def _norm(v):
            v = v.astype(np.float64)
            n = np.sqrt((v * v).sum(-1, keepdims=True))
            return v / np.maximum(n, 1e-12)

        a = _norm(np.asarray(normal_rebuild[b]))
        g = _norm(np.asarray(normal_gt[b]))
        t1n = g[idx1]
        t2n = a[idx2]
        nd1 = np.minimum(((a - t1n) ** 2).sum(-1), ((a + t1n) ** 2).sum(-1))
        nd2 = np.minimum(((g - t2n) ** 2).sum(-1), ((g + t2n) ** 2).sum(-1))
        loss_normal += nd1.mean() + nd2.mean()

    return (np.float32(loss_xyz / B), np.float32(loss_normal / B))


# revision 8
# speedup vs baseline: 1.4966x; 1.4966x over previous
"""Chamfer-distance-with-normals Trainium2 kernel.

Sharding: data-parallel over batch B=8 across the 8 NeuronCores (one batch
element per core). Per core, the 4096x4096 negated squared-distance matrix
W = -D is produced tile-by-tile by K=5 float32r matmuls that fold both
squared-norm rank-1 terms into the contraction:

    W[n, m] = 2*x1.x2 - |x2|^2 - |x1|^2  =  -D[n, m]

Per 128-row block, min-dist and argmin over the 4096-wide row come from two
single-pass scans (the NxM matrix is never materialized beyond one row-block):

  1. VectorE tensor_tensor_scan computes the running prefix-max S of W
     straight out of PSUM; its last column is -dist.
  2. ScalarE activation(Sign, bias=-rowmax, accum_out) counts the strictly-
     below-max prefix length: accum = -#{m : S[m] < rowmax} = -argmin
     (first-occurrence, exact -- S hits the max at the argmin and stays).

Two passes: [n,m] orientation for dist1/idx1 and [m,n] for dist2/idx2.
The O(B*N) normal gather + normalize + mean runs on host in float64.
"""

import functools
from contextlib import ExitStack

import numpy as np

import concourse.bass as bass
import concourse.mybir as mybir
import concourse.tile as tile
from concourse import bacc
from concourse.bass_utils import run_bass_kernel_spmd

B = 8
N_PTS = 4096
M_TILE = 512  # fp32 moving-operand max / one PSUM bank
P = 128


def build_chamfer(n_pts=N_PTS, m_tile=M_TILE, psum_halves=4):
    """Build the Bass program. Returns the compiled Bacc module."""
    nc = bacc.Bacc("TRN2", target_bir_lowering=False, debug=False, num_devices=B)
    dt = mybir.dt

    n_blocks = n_pts // P
    psum_halves = min(psum_halves, n_pts // m_tile)
    psum_w = psum_halves * m_tile
    n_chunks = n_pts // psum_w  # PSUM-width chunks per row-block

    ins = {}
    for name in ("s1", "t2", "s2", "t1"):
        ins[name] = nc.dram_tensor(name, [5, n_pts], dt.float32, kind="ExternalInput").ap()
    outs = {}
    for name in ("d1", "d2", "c1", "c2"):
        outs[name] = nc.dram_tensor(name, [P, n_blocks], dt.float32, kind="ExternalOutput").ap()

    with tile.TileContext(nc) as tc, ExitStack() as ctx:
        const_pool = ctx.enter_context(tc.tile_pool(name="const", bufs=1))
        s_pool = ctx.enter_context(tc.tile_pool(name="srow", bufs=2))
        psum_pool = ctx.enter_context(
            tc.tile_pool(name="psum", bufs=8 // psum_halves, space="PSUM")
        )
        res_pool = ctx.enter_context(tc.tile_pool(name="res", bufs=1))

        # Operands replicated at base partitions 0/32/64/96 so 4 matmuls can
        # run concurrently in distinct PE row-groups (K=5 << 128).
        n_grp = min(4, psum_halves * n_chunks)
        op_tiles = {}
        for name in ("s1", "t2", "s2", "t1"):
            t = const_pool.tile([32 * n_grp, n_pts], dt.float32, tag=name)
            for g in range(n_grp):
                nc.sync.dma_start(t[32 * g : 32 * g + 5, :], ins[name][:])
            op_tiles[name] = t
        dummy = const_pool.tile([P, psum_w], dt.float32, tag="dummy")
        nc.vector.memset(dummy[:], 0.0)
        junk = const_pool.tile([P, n_pts], dt.float32, tag="junk")

        for pno, (lhs_name, rhs_name, dname, cname) in enumerate(
            (("s1", "t2", "d1", "c1"), ("s2", "t1", "d2", "c2"))
        ):
            lhs = op_tiles[lhs_name]
            rhs = op_tiles[rhs_name]
            vals = res_pool.tile([P, n_blocks], dt.float32, tag=f"v{pno}")
            cnts = res_pool.tile([P, n_blocks], dt.float32, tag=f"c{pno}")

            for i in range(n_blocks):
                s_row = s_pool.tile([P, n_pts], dt.float32, tag="s")
                for h in range(n_chunks):
                    ps = psum_pool.tile([P, psum_w], dt.float32, tag="ps")
                    for jj in range(psum_halves):
                        j = psum_halves * h + jj
                        g = jj % n_grp
                        nc.tensor.matmul(
                            ps[:, jj * m_tile : (jj + 1) * m_tile],
                            lhs[32 * g : 32 * g + 5, i * P : (i + 1) * P],
                            rhs[32 * g : 32 * g + 5, j * m_tile : (j + 1) * m_tile],
                            start=True,
                            stop=True,
                            tile_position=(32 * g, 0),
                        )
                    # prefix max, chained across chunks via `initial`
                    nc.vector.tensor_tensor_scan(
                        s_row[:, h * psum_w : (h + 1) * psum_w],
                        ps[:],
                        dummy[:],
                        -3.0e38 if h == 0 else s_row[:, h * psum_w - 1 : h * psum_w],
                        op0=mybir.AluOpType.max,
                        op1=mybir.AluOpType.bypass,
                    )
                # dist = -rowmax (also the Sign bias below)
                nc.scalar.activation(
                    vals[:, i : i + 1],
                    s_row[:, n_pts - 1 : n_pts],
                    mybir.ActivationFunctionType.Copy,
                    scale=-1.0,
                )
                # accum = sum(sign(S - rowmax)) = -argmin (when sign(0)==0)
                nc.scalar.activation(
                    junk[:],
                    s_row[:],
                    mybir.ActivationFunctionType.Sign,
                    bias=vals[:, i : i + 1],
                    scale=1.0,
                    accum_out=cnts[:, i : i + 1],
                )

            nc.sync.dma_start(outs[dname][:], vals[:])
            nc.sync.dma_start(outs[cname][:], cnts[:])

    nc.compile()
    return nc


@functools.lru_cache(maxsize=1)
def _compiled():
    return build_chamfer()


def _operands(xyz):
    """[n,3] fp32 -> (S, T) [5,n] matmul operand rows (see module docstring)."""
    n = xyz.shape[0]
    x, y, z = xyz[:, 0], xyz[:, 1], xyz[:, 2]
    sq = x * x + y * y + z * z  # fp32
    ones = np.ones(n, np.float32)
    s = np.stack([x, y, z, ones, -sq]).astype(np.float32, copy=False)
    t = np.stack([2 * x, 2 * y, 2 * z, -sq, ones]).astype(np.float32, copy=False)
    return np.ascontiguousarray(s), np.ascontiguousarray(t)


def _decode_idx(counts, n_pts):
    """accum -> argmin index; handles sign(0) being 0 or +1 on HW."""
    if counts.max() > 0.5:  # sign(+0) == +1 convention
        idx = (n_pts - counts) * 0.5
    else:  # sign(0) == 0
        idx = -counts
    out = np.rint(idx).astype(np.int64)
    np.clip(out, 0, n_pts - 1, out=out)
    return out


def _unpack(res, n_pts):
    """Device outputs [128, blocks] -> flat [n] arrays (n = 128*block + p)."""
    d = {}
    for name in ("d1", "d2", "c1", "c2"):
        d[name] = np.ascontiguousarray(res[name].T).reshape(-1)
    return d


def kernel(xyz1, xyz2, normal_rebuild, normal_gt):
    nc = _compiled()

    in_maps = []
    for b in range(B):
        s1, t1 = _operands(np.asarray(xyz1[b], np.float32))
        s2, t2 = _operands(np.asarray(xyz2[b], np.float32))
        in_maps.append({"s1": s1, "t1": t1, "s2": s2, "t2": t2})

    res = run_bass_kernel_spmd(nc, in_maps, core_ids=list(range(B)))

    loss_xyz = 0.0
    loss_normal = 0.0
    for b in range(B):
        r = _unpack(res.results[b], N_PTS)
        idx1 = _decode_idx(r["c1"], N_PTS)
        idx2 = _decode_idx(r["c2"], N_PTS)
        # Exact distances at the device-selected neighbors (device min values
        # carry float32r matmul noise; the argmin itself is tie-robust).
        x1 = np.asarray(xyz1[b]).astype(np.float64)
        x2 = np.asarray(xyz2[b]).astype(np.float64)
        dist1 = ((x1 - x2[idx1]) ** 2).sum(-1)
        dist2 = ((x2 - x1[idx2]) ** 2).sum(-1)
        loss_xyz += dist1.mean() + dist2.mean()

        def _norm(v):
            v = v.astype(np.float64)
            n = np.sqrt((v * v).sum(-1, keepdims=True))
            return v / np.maximum(n, 1e-12)

        a = _norm(np.asarray(normal_rebuild[b]))
        g = _norm(np.asarray(normal_gt[b]))
        t1n = g[idx1]
        t2n = a[idx2]
        nd1 = np.minimum(((a - t1n) ** 2).sum(-1), ((a + t1n) ** 2).sum(-1))
        nd2 = np.minimum(((g - t2n) ** 2).sum(-1), ((g + t2n) ** 2).sum(-1))
        loss_normal += nd1.mean() + nd2.mean()

    return (np.float32(loss_xyz / B), np.float32(loss_normal / B))


# revision 11
# speedup vs baseline: 1.5332x; 1.0244x over previous
"""Chamfer-distance-with-normals Trainium2 kernel.

Sharding: data-parallel over batch B=8 across the 8 NeuronCores (one batch
element per core). Per core, the 4096x4096 negated squared-distance matrix
W = -D is produced one 128-row block at a time and never materialized:

    W[n, m] = 2*x1.x2 - |x2|^2 - |x1|^2  =  -D[n, m]

Matmuls use a bf16x3 decomposition (each fp32 operand split into three bf16
terms; the 6 dominant cross products + rank-1 norm terms give K=24 exact
bf16*bf16 products accumulated in fp32 PSUM, |W err| ~ 7e-6) so the PE runs
at 1 cycle/row instead of fp32's 4, packed 4x into 32-row PE groups.

Row argmin via two single-pass scans per 128-row block:
  1. VectorE tensor_tensor_scan runs a TWO-STREAM prefix max (data0 = PSUM
     half m in [0,2048), data1 = ScalarE-copied SBUF half [2048,4096)):
     state = max(max(state, lo[t]), hi[t]).  2048 steps cover 4096 columns.
  2. ScalarE activation(Sign, bias=-rowmax, accum_out) counts prefix entries
     strictly below the max: accum = -t* where t* is the first step at which
     the running max reaches the row max.  The argmin is then one of
     {t*, t*+2048}; the host resolves the pair (and computes exact distances
     and the O(B*N) normal losses) in float64.
"""

import functools
from contextlib import ExitStack

import ml_dtypes
import numpy as np

import concourse.bass as bass
import concourse.mybir as mybir
import concourse.tile as tile
from concourse import bacc
from concourse.bass_utils import run_bass_kernel_spmd

B = 8
N_PTS = 4096
P = 128
K_ROWS = 24  # bf16x3 decomposition rows


def build_chamfer(n_pts=N_PTS, m_tile=512):
    """Build the Bass program. Returns the compiled Bacc module."""
    nc = bacc.Bacc("TRN2", target_bir_lowering=False, debug=False, num_devices=B)
    dt = mybir.dt

    n_blocks = n_pts // P
    half = n_pts // 2
    m_tile = min(m_tile, half)
    m_tiles = half // m_tile  # matmuls per half
    n_grp = min(4, 2 * m_tiles)  # concurrent PE row-groups

    ins = {}
    for name in ("s1", "t2", "s2", "t1"):
        ins[name] = nc.dram_tensor(name, [K_ROWS, n_pts], dt.bfloat16, kind="ExternalInput").ap()
    outs = {}
    for name in ("d1", "d2", "c1", "c2"):
        outs[name] = nc.dram_tensor(name, [P, n_blocks], dt.float32, kind="ExternalOutput").ap()

    with tile.TileContext(nc) as tc, ExitStack() as ctx:
        const_pool = ctx.enter_context(tc.tile_pool(name="const", bufs=1))
        s_pool = ctx.enter_context(tc.tile_pool(name="srow", bufs=2))
        h1_pool = ctx.enter_context(tc.tile_pool(name="h1", bufs=2))
        psum_pool = ctx.enter_context(tc.tile_pool(name="psum", bufs=2, space="PSUM"))
        res_pool = ctx.enter_context(tc.tile_pool(name="res", bufs=1))

        # Operands replicated at base partitions 0/32/64/96 so up to 4 matmuls
        # run concurrently in distinct PE row-groups (K=24 <= 32).
        op_tiles = {}
        for name in ("s1", "t2", "s2", "t1"):
            t = const_pool.tile([32 * n_grp, n_pts], dt.bfloat16, tag=name)
            for g in range(n_grp):
                nc.sync.dma_start(t[32 * g : 32 * g + K_ROWS, :], ins[name][:])
            op_tiles[name] = t
        junk = const_pool.tile([P, half], dt.float32, tag="junk")

        for pno, (lhs_name, rhs_name, dname, cname) in enumerate(
            (("s1", "t2", "d1", "c1"), ("s2", "t1", "d2", "c2"))
        ):
            lhs = op_tiles[lhs_name]
            rhs = op_tiles[rhs_name]
            vals = res_pool.tile([P, n_blocks], dt.float32, tag=f"v{pno}")
            cnts = res_pool.tile([P, n_blocks], dt.float32, tag=f"c{pno}")

            for i in range(n_blocks):
                ps0 = psum_pool.tile([P, half], dt.float32, tag="ps")
                ps1 = psum_pool.tile([P, half], dt.float32, tag="ps")
                ps = (ps0, ps1)
                for h in range(2):
                    for jj in range(m_tiles):
                        j = h * m_tiles + jj
                        g = (h * m_tiles + jj) % n_grp
                        nc.tensor.matmul(
                            ps[h][:, jj * m_tile : (jj + 1) * m_tile],
                            lhs[32 * g : 32 * g + K_ROWS, i * P : (i + 1) * P],
                            rhs[32 * g : 32 * g + K_ROWS, j * m_tile : (j + 1) * m_tile],
                            start=True,
                            stop=True,
                            tile_position=(32 * g, 0),
                        )
                # upper half to SBUF (scan streams can't both be PSUM)
                h1_sb = h1_pool.tile([P, half], dt.float32, tag="h1")
                nc.scalar.copy(h1_sb[:], ps[1][:])
                # two-stream prefix max over (lo[t], hi[t]) pairs
                s_row = s_pool.tile([P, half], dt.float32, tag="s")
                nc.vector.tensor_tensor_scan(
                    s_row[:],
                    ps[0][:],
                    h1_sb[:],
                    -3.0e38,
                    op0=mybir.AluOpType.max,
                    op1=mybir.AluOpType.max,
                )
                # rowmax (negated: = +min dist, and the Sign bias below)
                nc.scalar.activation(
                    vals[:, i : i + 1],
                    s_row[:, half - 1 : half],
                    mybir.ActivationFunctionType.Copy,
                    scale=-1.0,
                )
                # accum = sum(sign(S - rowmax)) = -t* (when sign(0)==0)
                nc.scalar.activation(
                    junk[:],
                    s_row[:],
                    mybir.ActivationFunctionType.Sign,
                    bias=vals[:, i : i + 1],
                    scale=1.0,
                    accum_out=cnts[:, i : i + 1],
                )

            nc.sync.dma_start(outs[dname][:], vals[:])
            nc.sync.dma_start(outs[cname][:], cnts[:])

    nc.compile()
    return nc


@functools.lru_cache(maxsize=1)
def _compiled():
    return build_chamfer()


def _bf3(v):
    h = v.astype(ml_dtypes.bfloat16).astype(np.float32)
    r = v - h
    m = r.astype(ml_dtypes.bfloat16).astype(np.float32)
    l = (r - m).astype(ml_dtypes.bfloat16)
    return h.astype(ml_dtypes.bfloat16), m.astype(ml_dtypes.bfloat16), l


def _operands(xyz):
    """[n,3] fp32 -> (S, T) [24,n] bf16 stationary/moving operand rows.

    Row pairing (S row k multiplies T row k): per dim the 6 dominant bf16x3
    cross terms (hh, hm, mh, hl, lh, mm), then ones x (-sq h/m/l) and
    (-sq h/m/l) x ones.
    """
    n = xyz.shape[0]
    x32 = xyz.astype(np.float32)
    sq = (x32 * x32).sum(1)
    ones = np.ones(n, ml_dtypes.bfloat16)
    s_rows, t_rows = [], []
    for d in range(3):
        ah, am, al = _bf3(x32[:, d])
        bh, bm, bl = _bf3(2.0 * x32[:, d])
        s_rows += [ah, ah, am, ah, al, am]
        t_rows += [bh, bm, bh, bl, bh, bm]
    nh, nm, nl = _bf3(-sq)
    s_rows += [ones, ones, ones, nh, nm, nl]
    t_rows += [nh, nm, nl, ones, ones, ones]
    return (
        np.ascontiguousarray(np.stack(s_rows)),
        np.ascontiguousarray(np.stack(t_rows)),
    )


def _decode_step(counts, half):
    """accum -> first-step index t*; handles sign(0) being 0 or +1 on HW."""
    if counts.max() > 0.5:  # sign(+0) == +1 convention
        t = (half - counts) * 0.5
    else:  # sign(0) == 0
        t = -counts
    out = np.rint(t).astype(np.int64)
    np.clip(out, 0, half - 1, out=out)
    return out


def _resolve_idx(tstar, xa, xb, half):
    """Pick the true argmin among candidates {t*, t*+half} by exact distance."""
    c0 = ((xa - xb[tstar]) ** 2).sum(-1)
    c1 = ((xa - xb[tstar + half]) ** 2).sum(-1)
    take_hi = c1 < c0
    idx = np.where(take_hi, tstar + half, tstar)
    dist = np.where(take_hi, c1, c0)
    return idx, dist


def kernel(xyz1, xyz2, normal_rebuild, normal_gt):
    nc = _compiled()

    in_maps = []
    for b in range(B):
        s1, t1 = _operands(np.asarray(xyz1[b], np.float32))
        s2, t2 = _operands(np.asarray(xyz2[b], np.float32))
        in_maps.append({"s1": s1, "t1": t1, "s2": s2, "t2": t2})

    res = run_bass_kernel_spmd(nc, in_maps, core_ids=list(range(B)))

    half = N_PTS // 2
    loss_xyz = 0.0
    loss_normal = 0.0
    for b in range(B):
        r = {k: np.ascontiguousarray(v.T).reshape(-1) for k, v in res.results[b].items()}
        t1s = _decode_step(r["c1"], half)
        t2s = _decode_step(r["c2"], half)
        x1 = np.asarray(xyz1[b]).astype(np.float64)
        x2 = np.asarray(xyz2[b]).astype(np.float64)
        idx1, dist1 = _resolve_idx(t1s, x1, x2, half)
        idx2, dist2 = _resolve_idx(t2s, x2, x1, half)
        loss_xyz += dist1.mean() + dist2.mean()

        def _norm(v):
            v = v.astype(np.float64)
            n = np.sqrt((v * v).sum(-1, keepdims=True))
            return v / np.maximum(n, 1e-12)

        a = _norm(np.asarray(normal_rebuild[b]))
        g = _norm(np.asarray(normal_gt[b]))
        t1n = g[idx1]
        t2n = a[idx2]
        nd1 = np.minimum(((a - t1n) ** 2).sum(-1), ((a + t1n) ** 2).sum(-1))
        nd2 = np.minimum(((g - t2n) ** 2).sum(-1), ((g + t2n) ** 2).sum(-1))
        loss_normal += nd1.mean() + nd2.mean()

    return (np.float32(loss_xyz / B), np.float32(loss_normal / B))


# revision 16
# speedup vs baseline: 1.9566x; 1.2762x over previous
"""Chamfer-distance-with-normals Trainium2 kernel.

Sharding: data-parallel over batch B=8 across the 8 NeuronCores (one batch
element per core). Per core, the 4096x4096 negated squared-distance matrix
W = -D is produced one 128-row block at a time and never materialized:

    W[n, m] = 2*x1.x2 - |x2|^2 - |x1|^2  =  -D[n, m]

Matmuls use a bf16x3 decomposition (each fp32 operand split into three bf16
terms; the 6 dominant cross products + rank-1 norm terms give K=24 exact
bf16*bf16 products accumulated in fp32 PSUM, |W err| ~ 7e-6) so the PE runs
at 1 cycle/row instead of fp32's 4, packed 4x into 32-row PE groups.

Row argmin via two single-pass scans per 128-row block:
  1. VectorE tensor_tensor_scan runs a TWO-STREAM prefix max (data0 = PSUM
     half m in [0,2048), data1 = ScalarE-copied SBUF half [2048,4096)):
     state = max(max(state, lo[t]), hi[t]).  2048 steps cover 4096 columns.
  2. ScalarE activation(Sign, bias=-rowmax, accum_out) counts prefix entries
     strictly below the max: accum = -t* where t* is the first step at which
     the running max reaches the row max.  The argmin is then one of
     {t*, t*+2048}; the host resolves the pair (and computes exact distances
     and the O(B*N) normal losses) in float64.
"""

import functools
from contextlib import ExitStack

import ml_dtypes
import numpy as np

import concourse.bass as bass
import concourse.mybir as mybir
import concourse.tile as tile
from concourse import bacc
from concourse.bass_utils import run_bass_kernel_spmd

B = 8
N_PTS = 4096
P = 128
K_ROWS = 24  # bf16x3 decomposition rows


def build_chamfer(n_pts=N_PTS, m_tile=512):
    """Build the Bass program. Returns the compiled Bacc module."""
    nc = bacc.Bacc("TRN2", target_bir_lowering=False, debug=False, num_devices=B)
    dt = mybir.dt

    n_blocks = n_pts // P
    half = n_pts // 2
    quart = half // 2  # one PSUM tile / one scan chunk
    m_tile = min(m_tile, quart)
    m_tiles = quart // m_tile  # matmuls per quarter
    n_grp = min(4, 4 * m_tiles)  # concurrent PE row-groups

    ins = {}
    for name in ("s1", "t2", "s2", "t1"):
        ins[name] = nc.dram_tensor(name, [K_ROWS, n_pts], dt.bfloat16, kind="ExternalInput").ap()
    outs = {}
    for name in ("c1", "c2"):
        outs[name] = nc.dram_tensor(name, [P, n_blocks], dt.float32, kind="ExternalOutput").ap()

    with tile.TileContext(nc) as tc, ExitStack() as ctx:
        const_pool = ctx.enter_context(tc.tile_pool(name="const", bufs=1))
        s_pool = ctx.enter_context(tc.tile_pool(name="srow", bufs=2))
        h1_pool = ctx.enter_context(tc.tile_pool(name="h1", bufs=2))
        psum_pool = ctx.enter_context(tc.tile_pool(name="psum", bufs=4, space="PSUM"))
        res_pool = ctx.enter_context(tc.tile_pool(name="res", bufs=1))

        # Operands replicated at base partitions 0/32/64/96 so up to 4 matmuls
        # run concurrently in distinct PE row-groups (K=24 <= 32).
        op_tiles = {}
        for name in ("s1", "t2", "s2", "t1"):
            t = const_pool.tile([32 * n_grp, n_pts], dt.bfloat16, tag=name)
            for g in range(n_grp):
                nc.sync.dma_start(t[32 * g : 32 * g + K_ROWS, :], ins[name][:])
            op_tiles[name] = t
        junk = const_pool.tile([P, half], dt.float32, tag="junk")

        def emit_count(s_row, cnt_ap):
            # accum = sum(sign(rowmin - S)) = -t* (when sign(0)==0); the bias
            # reads the row min straight from the scan's last column.
            nc.scalar.activation(
                junk[:],
                s_row[:],
                mybir.ActivationFunctionType.Sign,
                bias=s_row[:, half - 1 : half],
                scale=-1.0,
                accum_out=cnt_ap,
            )

        for pno, (lhs_name, rhs_name, cname) in enumerate(
            (("s1", "t2", "c1"), ("s2", "t1", "c2"))
        ):
            lhs = op_tiles[lhs_name]
            rhs = op_tiles[rhs_name]
            cnts = res_pool.tile([P, n_blocks], dt.float32, tag=f"c{pno}")

            pending = None  # (s_row, count slice) lagging one block
            for i in range(n_blocks):
                # 4 PSUM quarter-tiles: lo half scanned from PSUM, hi half
                # copied to SBUF (the scan's two streams can't both be PSUM).
                qs = []
                for q in range(4):
                    pq = psum_pool.tile([P, quart], dt.float32, tag="ps")
                    qs.append(pq)
                    for jj in range(m_tiles):
                        j = q * m_tiles + jj
                        g = j % n_grp
                        nc.tensor.matmul(
                            pq[:, jj * m_tile : (jj + 1) * m_tile],
                            lhs[32 * g : 32 * g + K_ROWS, i * P : (i + 1) * P],
                            rhs[32 * g : 32 * g + K_ROWS, j * m_tile : (j + 1) * m_tile],
                            start=True,
                            stop=True,
                            tile_position=(32 * g, 0),
                        )
                h1_sb = h1_pool.tile([P, half], dt.float32, tag="h1")
                nc.scalar.copy(h1_sb[:, :quart], qs[2][:])
                nc.scalar.copy(h1_sb[:, quart:], qs[3][:])
                # two-stream prefix min over (lo[t], hi[t]=lo[t]+half) pairs
                s_row = s_pool.tile([P, half], dt.float32, tag="s")
                nc.vector.tensor_tensor_scan(
                    s_row[:, :quart],
                    qs[0][:],
                    h1_sb[:, :quart],
                    3.0e38,
                    op0=mybir.AluOpType.min,
                    op1=mybir.AluOpType.min,
                )
                nc.vector.tensor_tensor_scan(
                    s_row[:, quart:],
                    qs[1][:],
                    h1_sb[:, quart:],
                    s_row[:, quart - 1 : quart],
                    op0=mybir.AluOpType.min,
                    op1=mybir.AluOpType.min,
                )
                if pending is not None:
                    emit_count(*pending)
                pending = (s_row, cnts[:, i : i + 1])
            emit_count(*pending)

            nc.sync.dma_start(outs[cname][:], cnts[:])

    nc.compile()
    return nc


@functools.lru_cache(maxsize=1)
def _compiled():
    return build_chamfer()


def _bf3(v):
    h = v.astype(ml_dtypes.bfloat16).astype(np.float32)
    r = v - h
    m = r.astype(ml_dtypes.bfloat16).astype(np.float32)
    l = (r - m).astype(ml_dtypes.bfloat16)
    return h.astype(ml_dtypes.bfloat16), m.astype(ml_dtypes.bfloat16), l


def _operands(xyz):
    """[n,3] fp32 -> (S, T) [24,n] bf16 stationary/moving operand rows.

    Row pairing (S row k multiplies T row k): per dim the 6 dominant bf16x3
    cross terms (hh, hm, mh, hl, lh, mm), then ones x (-sq h/m/l) and
    (-sq h/m/l) x ones.
    """
    n = xyz.shape[0]
    x32 = xyz.astype(np.float32)
    sq = (x32 * x32).sum(1)
    ones = np.ones(n, ml_dtypes.bfloat16)
    s_rows, t_rows = [], []
    for d in range(3):
        ah, am, al = _bf3(x32[:, d])
        bh, bm, bl = _bf3(-2.0 * x32[:, d])
        s_rows += [ah, ah, am, ah, al, am]
        t_rows += [bh, bm, bh, bl, bh, bm]
    nh, nm, nl = _bf3(sq)
    s_rows += [ones, ones, ones, nh, nm, nl]
    t_rows += [nh, nm, nl, ones, ones, ones]
    return (
        np.ascontiguousarray(np.stack(s_rows)),
        np.ascontiguousarray(np.stack(t_rows)),
    )


def _decode_step(counts, half):
    """accum -> first-step index t*; handles sign(0) being 0 or +1 on HW."""
    if counts.max() > 0.5:  # sign(+0) == +1 convention
        t = (half - counts) * 0.5
    else:  # sign(0) == 0
        t = -counts
    out = np.rint(t).astype(np.int64)
    np.clip(out, 0, half - 1, out=out)
    return out


def _resolve_idx(tstar, xa, xb, half):
    """Pick the true argmin among candidates {t*, t*+half} by exact distance."""
    c0 = ((xa - xb[tstar]) ** 2).sum(-1)
    c1 = ((xa - xb[tstar + half]) ** 2).sum(-1)
    take_hi = c1 < c0
    idx = np.where(take_hi, tstar + half, tstar)
    dist = np.where(take_hi, c1, c0)
    return idx, dist


def kernel(xyz1, xyz2, normal_rebuild, normal_gt):
    nc = _compiled()

    in_maps = []
    for b in range(B):
        s1, t1 = _operands(np.asarray(xyz1[b], np.float32))
        s2, t2 = _operands(np.asarray(xyz2[b], np.float32))
        in_maps.append({"s1": s1, "t1": t1, "s2": s2, "t2": t2})

    res = run_bass_kernel_spmd(nc, in_maps, core_ids=list(range(B)))

    half = N_PTS // 2
    loss_xyz = 0.0
    loss_normal = 0.0
    for b in range(B):
        r = {k: np.ascontiguousarray(v.T).reshape(-1) for k, v in res.results[b].items()}
        t1s = _decode_step(r["c1"], half)
        t2s = _decode_step(r["c2"], half)
        x1 = np.asarray(xyz1[b]).astype(np.float64)
        x2 = np.asarray(xyz2[b]).astype(np.float64)
        idx1, dist1 = _resolve_idx(t1s, x1, x2, half)
        idx2, dist2 = _resolve_idx(t2s, x2, x1, half)
        loss_xyz += dist1.mean() + dist2.mean()

        def _norm(v):
            v = v.astype(np.float64)
            n = np.sqrt((v * v).sum(-1, keepdims=True))
            return v / np.maximum(n, 1e-12)

        a = _norm(np.asarray(normal_rebuild[b]))
        g = _norm(np.asarray(normal_gt[b]))
        t1n = g[idx1]
        t2n = a[idx2]
        nd1 = np.minimum(((a - t1n) ** 2).sum(-1), ((a + t1n) ** 2).sum(-1))
        nd2 = np.minimum(((g - t2n) ** 2).sum(-1), ((g + t2n) ** 2).sum(-1))
        loss_normal += nd1.mean() + nd2.mean()

    return (np.float32(loss_xyz / B), np.float32(loss_normal / B))


# revision 20
# speedup vs baseline: 2.4777x; 1.2663x over previous
"""Chamfer-distance-with-normals Trainium2 kernel.

Sharding: data-parallel over batch B=8 across the 8 NeuronCores (one batch
element per core). Per core, the 4096x4096 negated squared-distance matrix
W = -D is produced one 128-row block at a time and never materialized:

    W[n, m] = 2*x1.x2 - |x2|^2 - |x1|^2  =  -D[n, m]

Matmuls use a bf16x3 decomposition (each fp32 operand split into three bf16
terms; the 6 dominant cross products + rank-1 norm terms give K=24 exact
bf16*bf16 products accumulated in fp32 PSUM, |W err| ~ 7e-6) so the PE runs
at 1 cycle/row instead of fp32's 4, packed 4x into 32-row PE groups.

Row argmin via two single-pass scans per 128-row block:
  1. VectorE tensor_tensor_scan runs a TWO-STREAM prefix max (data0 = PSUM
     half m in [0,2048), data1 = ScalarE-copied SBUF half [2048,4096)):
     state = max(max(state, lo[t]), hi[t]).  2048 steps cover 4096 columns.
  2. ScalarE activation(Sign, bias=-rowmax, accum_out) counts prefix entries
     strictly below the max: accum = -t* where t* is the first step at which
     the running max reaches the row max.  The argmin is then one of
     {t*, t*+2048}; the host resolves the pair (and computes exact distances
     and the O(B*N) normal losses) in float64.
"""

import functools
from contextlib import ExitStack

import ml_dtypes
import numpy as np

import concourse.bass as bass
import concourse.mybir as mybir
import concourse.tile as tile
from concourse import bacc
from concourse.bass_utils import run_bass_kernel_spmd

B = 8
N_PTS = 4096
P = 128
K_ROWS = 24  # bf16x3 decomposition rows


def build_chamfer(n_pts=N_PTS, m_tile=512):
    """Build the Bass program. Returns the compiled Bacc module."""
    nc = bacc.Bacc("TRN2", target_bir_lowering=False, debug=False, num_devices=B)
    dt = mybir.dt

    n_blocks = n_pts // P
    half = n_pts // 2
    m_tile = min(m_tile, half)
    m_tiles = half // m_tile  # matmuls per half
    n_grp = min(4, m_tiles)  # concurrent PE row-groups

    ins = {}
    for name in ("s1", "t2", "s2", "t1"):
        ins[name] = nc.dram_tensor(name, [K_ROWS, n_pts], dt.bfloat16, kind="ExternalInput").ap()
    outs = {}
    for name in ("c1", "c2"):
        outs[name] = nc.dram_tensor(name, [P, n_blocks], dt.float32, kind="ExternalOutput").ap()

    with tile.TileContext(nc) as tc, ExitStack() as ctx:
        const_pool = ctx.enter_context(tc.tile_pool(name="const", bufs=1))
        s_pool = ctx.enter_context(tc.tile_pool(name="srow", bufs=2))
        h1_pool = ctx.enter_context(tc.tile_pool(name="h1", bufs=2))
        pslo_pool = ctx.enter_context(tc.tile_pool(name="pslo", bufs=1, space="PSUM"))
        pshi_pool = ctx.enter_context(tc.tile_pool(name="pshi", bufs=1, space="PSUM"))
        res_pool = ctx.enter_context(tc.tile_pool(name="res", bufs=1))

        # Operands replicated at base partitions 0/32/64/96 so up to 4 matmuls
        # run concurrently in distinct PE row-groups (K=24 <= 32).
        op_tiles = {}
        for name in ("s1", "t2", "s2", "t1"):
            t = const_pool.tile([32 * n_grp, n_pts], dt.bfloat16, tag=name)
            for g in range(n_grp):
                nc.sync.dma_start(t[32 * g : 32 * g + K_ROWS, :], ins[name][:])
            op_tiles[name] = t
        junk = const_pool.tile([P, half], dt.float32, tag="junk")

        def emit_count(s_row, cnt_ap):
            # accum = sum(sign(rowmin - S)) = -t* (when sign(0)==0); the bias
            # reads the row min straight from the scan's last column.
            nc.scalar.activation(
                junk[:],
                s_row[:],
                mybir.ActivationFunctionType.Sign,
                bias=s_row[:, half - 1 : half],
                scale=-1.0,
                accum_out=cnt_ap,
            )

        for pno, (lhs_name, rhs_name, cname) in enumerate(
            (("s1", "t2", "c1"), ("s2", "t1", "c2"))
        ):
            lhs = op_tiles[lhs_name]
            rhs = op_tiles[rhs_name]
            cnts = res_pool.tile([P, n_blocks], dt.float32, tag=f"c{pno}")

            def mm_half(ps, i, h):
                for jj in range(m_tiles):
                    j = h * m_tiles + jj
                    g = jj % n_grp
                    nc.tensor.matmul(
                        ps[:, jj * m_tile : (jj + 1) * m_tile],
                        lhs[32 * g : 32 * g + K_ROWS, i * P : (i + 1) * P],
                        rhs[32 * g : 32 * g + K_ROWS, j * m_tile : (j + 1) * m_tile],
                        start=True,
                        stop=True,
                        tile_position=(32 * g, 0),
                    )

            pending = None  # (s_row, count slice) lagging one block
            for i in range(n_blocks):
                # hi half first: its PSUM bank recycles via the early ACT
                # copy, so next block's matmuls never wait on the scan.
                ps_hi = pshi_pool.tile([P, half], dt.float32, tag="ph")
                mm_half(ps_hi, i, 1)
                h1_sb = h1_pool.tile([P, half], dt.float32, tag="h1")
                nc.scalar.copy(h1_sb[:], ps_hi[:])
                ps_lo = pslo_pool.tile([P, half], dt.float32, tag="pl")
                mm_half(ps_lo, i, 0)
                # two-stream prefix min over (lo[t], hi[t]=lo[t]+half) pairs
                s_row = s_pool.tile([P, half], dt.float32, tag="s")
                nc.vector.tensor_tensor_scan(
                    s_row[:],
                    ps_lo[:],
                    h1_sb[:],
                    3.0e38,
                    op0=mybir.AluOpType.min,
                    op1=mybir.AluOpType.min,
                )
                if pending is not None:
                    emit_count(*pending)
                pending = (s_row, cnts[:, i : i + 1])
            emit_count(*pending)

            nc.sync.dma_start(outs[cname][:], cnts[:])

    nc.compile()
    return nc


@functools.lru_cache(maxsize=1)
def _compiled():
    return build_chamfer()


def _bf3(v):
    h = v.astype(ml_dtypes.bfloat16).astype(np.float32)
    r = v - h
    m = r.astype(ml_dtypes.bfloat16).astype(np.float32)
    l = (r - m).astype(ml_dtypes.bfloat16)
    return h.astype(ml_dtypes.bfloat16), m.astype(ml_dtypes.bfloat16), l


def _operands(xyz):
    """[n,3] fp32 -> (S, T) [24,n] bf16 stationary/moving operand rows.

    Row pairing (S row k multiplies T row k): per dim the 6 dominant bf16x3
    cross terms (hh, hm, mh, hl, lh, mm), then ones x (-sq h/m/l) and
    (-sq h/m/l) x ones.
    """
    n = xyz.shape[0]
    x32 = xyz.astype(np.float32)
    sq = (x32 * x32).sum(1)
    ones = np.ones(n, ml_dtypes.bfloat16)
    s_rows, t_rows = [], []
    for d in range(3):
        ah, am, al = _bf3(x32[:, d])
        bh, bm, bl = _bf3(-2.0 * x32[:, d])
        s_rows += [ah, ah, am, ah, al, am]
        t_rows += [bh, bm, bh, bl, bh, bm]
    nh, nm, nl = _bf3(sq)
    s_rows += [ones, ones, ones, nh, nm, nl]
    t_rows += [nh, nm, nl, ones, ones, ones]
    return (
        np.ascontiguousarray(np.stack(s_rows)),
        np.ascontiguousarray(np.stack(t_rows)),
    )


def _decode_step(counts, half):
    """accum -> first-step index t*; handles sign(0) being 0 or +1 on HW."""
    if counts.max() > 0.5:  # sign(+0) == +1 convention
        t = (half - counts) * 0.5
    else:  # sign(0) == 0
        t = -counts
    out = np.rint(t).astype(np.int64)
    np.clip(out, 0, half - 1, out=out)
    return out


def _resolve_idx(tstar, xa, xb, half):
    """Pick the true argmin among candidates {t*, t*+half} by exact distance."""
    c0 = ((xa - xb[tstar]) ** 2).sum(-1)
    c1 = ((xa - xb[tstar + half]) ** 2).sum(-1)
    take_hi = c1 < c0
    idx = np.where(take_hi, tstar + half, tstar)
    dist = np.where(take_hi, c1, c0)
    return idx, dist


def kernel(xyz1, xyz2, normal_rebuild, normal_gt):
    nc = _compiled()

    in_maps = []
    for b in range(B):
        s1, t1 = _operands(np.asarray(xyz1[b], np.float32))
        s2, t2 = _operands(np.asarray(xyz2[b], np.float32))
        in_maps.append({"s1": s1, "t1": t1, "s2": s2, "t2": t2})

    res = run_bass_kernel_spmd(nc, in_maps, core_ids=list(range(B)))

    half = N_PTS // 2
    loss_xyz = 0.0
    loss_normal = 0.0
    for b in range(B):
        r = {k: np.ascontiguousarray(v.T).reshape(-1) for k, v in res.results[b].items()}
        t1s = _decode_step(r["c1"], half)
        t2s = _decode_step(r["c2"], half)
        x1 = np.asarray(xyz1[b]).astype(np.float64)
        x2 = np.asarray(xyz2[b]).astype(np.float64)
        idx1, dist1 = _resolve_idx(t1s, x1, x2, half)
        idx2, dist2 = _resolve_idx(t2s, x2, x1, half)
        loss_xyz += dist1.mean() + dist2.mean()

        def _norm(v):
            v = v.astype(np.float64)
            n = np.sqrt((v * v).sum(-1, keepdims=True))
            return v / np.maximum(n, 1e-12)

        a = _norm(np.asarray(normal_rebuild[b]))
        g = _norm(np.asarray(normal_gt[b]))
        t1n = g[idx1]
        t2n = a[idx2]
        nd1 = np.minimum(((a - t1n) ** 2).sum(-1), ((a + t1n) ** 2).sum(-1))
        nd2 = np.minimum(((g - t2n) ** 2).sum(-1), ((g + t2n) ** 2).sum(-1))
        loss_normal += nd1.mean() + nd2.mean()

    return (np.float32(loss_xyz / B), np.float32(loss_normal / B))
